# revision 36
# baseline (speedup 1.0000x reference)
"""Causal self-attention with RoPE, sharded over 8 TRN2 NeuronCores.

Sharding: data-parallel over B (4 ways) x tensor-parallel over heads
(2 ways, 6 heads each). Each core computes qkv projection, RoPE,
causal attention and a partial output projection for its (batch,
head-half); the host sums the two head-half partials per batch.

Attention processes heads in PAIRS using PE-array row tiling: the
score matmuls contract over d=64, so head 2c runs in PE rows 0:63
and head 2c+1 in rows 64:127 concurrently (tile_position derived
from base partitions) - 2x effective score throughput vs the
zero-padded K=128 form. The causal mask is applied post-exp as a
DVE multiply with a 0/1 lower-triangular matrix on the diagonal
128x128 blocks (no mask matmuls). V carries a ones column (M=65
stationary) so PV also yields the softmax denominator.

RoPE pairs are host-permuted into contiguous even/odd halves per head
(legal because the QK contraction is invariant to permuting head dims
as long as q and k share the permutation, and V is not roped).
"""

import numpy as np
import ml_dtypes

import concourse.bass as bass
import concourse.tile as tile
import concourse.mybir as mybir
from concourse import bacc
from concourse.bass_utils import run_bass_kernel_spmd

B, T, C, H, D = 4, 2048, 768, 12, 64
HL = H // 2          # heads per core
TB = T // 128        # 16 t-blocks
CB = C // 128        # 6 contraction blocks
NCORES = 8

F32 = mybir.dt.float32
BF16 = mybir.dt.bfloat16
AF = mybir.ActivationFunctionType

_CACHED_NC = None


def build_nc():
    nc = bacc.Bacc("TRN2", target_bir_lowering=False)

    xT = nc.declare_dram_parameter("xT", [C, T], BF16, isOutput=False)
    wqkvT = nc.declare_dram_parameter("wqkvT", [C, 3 * HL * D], BF16, isOutput=False)
    wpT = nc.declare_dram_parameter("wpT", [HL * D, C], BF16, isOutput=False)
    sinr = nc.declare_dram_parameter("sinr", [T, HL * D], BF16, isOutput=False)
    cosr = nc.declare_dram_parameter("cosr", [T, HL * D], BF16, isOutput=False)
    negid = nc.declare_dram_parameter("negid", [128, 128], BF16, isOutput=False)
    triu512 = nc.declare_dram_parameter("triu512", [128, 512], BF16, isOutput=False)
    ident = nc.declare_dram_parameter("ident", [128, 128], BF16, isOutput=False)
    out = nc.declare_dram_parameter("out", [T, C], BF16, isOutput=True)

    with tile.TileContext(nc) as tc:
        with (
            tc.tile_pool(name="persist", bufs=1) as persist,
            tc.tile_pool(name="pPp", bufs=10) as pPp,
            tc.tile_pool(name="small", bufs=4) as small,
        ):
            # ---- persistent SBUF tensors ----
            # qT_all: head-pair block hb occupies cols [hb*2048, (hb+1)*2048);
            # head h lives at rows (h%2)*64 of block h//2.
            qT_all = persist.tile([128, 3 * T], BF16, tag="qT", name="qT_all")
            # kTp_all: head h occupies cols [h*2048, ...), rows (h%2)*64.
            # The other 64 rows of each col block are never read (K=64
            # row-tiled score matmuls).
            kTp_all = persist.tile([128, HL * T], BF16, tag="kT", name="kTp_all")
            # v tiles: per t-block, head h at cols [h*128, h*128+64] = v,
            # col h*128+64 = ones; cols beyond 65 are never read (M=65).
            v_sb = [persist.tile([128, HL, 128], BF16, tag=f"v{i}", name=f"v{i}")
                    for i in range(TB)]
            wp_sb = [persist.tile([128, C], BF16, tag=f"wp{i}", name=f"wp{i}")
                     for i in range(3)]
            yT64 = [persist.tile([64, T], BF16, tag=f"y64_{i}", name=f"y64_{i}")
                    for i in range(HL)]
            yT128 = [persist.tile([128, T], BF16, tag=f"y128_{i}", name=f"y128_{i}")
                     for i in range(3)]
            negid_sb = persist.tile([128, 128], BF16, tag="negid")
            triu_sb = persist.tile([128, 512], BF16, tag="triu512")
            id_sb = persist.tile([128, 128], BF16, tag="ident")

            nc.gpsimd.dma_start(out=negid_sb, in_=negid[:, :])
            nc.gpsimd.dma_start(out=triu_sb, in_=triu512[:, :])
            nc.gpsimd.dma_start(out=id_sb, in_=ident[:, :])
            for i in range(3):
                nc.gpsimd.dma_start(out=wp_sb[i], in_=wpT[i * 128:(i + 1) * 128, :])

            kTp_v = kTp_all.rearrange("p (c two t) -> p c two t", c=3, two=2)

            # ================= phase 1: qkv + rope + transpose =================
            with (
                tc.tile_pool(name="p1in", bufs=1) as p1in,
                tc.tile_pool(name="p1work", bufs=3) as p1w,
                tc.tile_pool(name="p1psum", bufs=6, space="PSUM") as p1ps,
                tc.tile_pool(name="ptpsum", bufs=2, space="PSUM") as ptps,
            ):
                xT_sb = [p1in.tile([128, T], BF16, tag=f"xT{i}", name=f"xTs{i}")
                         for i in range(CB)]
                w_sb = [p1in.tile([128, 3 * HL * D], BF16, tag=f"w{i}", name=f"ws{i}")
                        for i in range(CB)]
                for i in range(CB):
                    # stage the first t-blocks / q-chunk first so tb=0's qkv
                    # matmuls start while the bulk still streams; x on the
                    # act issue queue, w on sync, so staging issues in
                    # parallel (act is idle this early)
                    nc.scalar.dma_start(out=xT_sb[i][:, 0:512],
                                        in_=xT[i * 128:(i + 1) * 128, 0:512])
                    nc.sync.dma_start(out=w_sb[i][:, 0:384],
                                      in_=wqkvT[i * 128:(i + 1) * 128, 0:384])
                for i in range(CB):
                    nc.scalar.dma_start(out=xT_sb[i][:, 512:T],
                                        in_=xT[i * 128:(i + 1) * 128, 512:T])
                    nc.sync.dma_start(out=w_sb[i][:, 384:],
                                      in_=wqkvT[i * 128:(i + 1) * 128, 384:])

                for tb in range(TB):
                    if tb == 1:
                        # one-time ones column for the denominator trick,
                        # emitted here so it overlaps phase 1 instead of
                        # gating tb=0
                        for i in range(TB):
                            nc.gpsimd.memset(v_sb[i][:, :, D:D + 1], 1.0)
                    tsl = bass.ts(tb, 128)
                    ps_qkv = []
                    for chunk in range(3):
                        ps = p1ps.tile([128, 384], F32, tag="qkv", name="psqkv")
                        for cb in range(CB):
                            nc.tensor.matmul(
                                ps,
                                lhsT=xT_sb[cb][:, tsl],
                                rhs=w_sb[cb][:, chunk * 384:(chunk + 1) * 384],
                                start=(cb == 0),
                                stop=(cb == CB - 1),
                            )
                        ps_qkv.append(ps)

                    sin_sb = p1w.tile([128, HL * D], BF16, tag="sin", name="sin_sb")
                    cos_sb = p1w.tile([128, HL * D], BF16, tag="cos", name="cos_sb")
                    nc.gpsimd.dma_start(out=sin_sb, in_=sinr[tsl, :])
                    nc.gpsimd.dma_start(out=cos_sb, in_=cosr[tsl, :])

                    # v: pack t-major (ones col already set)
                    nc.scalar.copy(
                        v_sb[tb][:, :, 0:D],
                        ps_qkv[2].rearrange("p (h d) -> p h d", h=HL),
                    )

                    # rope directly from PSUM; halves layout (host-permuted):
                    # per head cols [evens(32) | odds(32)]
                    for qk in range(2):
                        ro = p1w.tile([128, HL * D], BF16, tag=f"ro{qk}",
                                      name=f"ro{qk}")
                        t1 = p1w.tile([128, HL * D], BF16, tag="t1", name="t1")
                        t2 = p1w.tile([128, HL * D], BF16, tag="t2", name="t2")
                        nc.vector.tensor_mul(t1, ps_qkv[qk], cos_sb)
                        nc.vector.tensor_mul(t2, ps_qkv[qk], sin_sb)
                        rv = ro.rearrange("p (h half i) -> p h half i", h=HL, half=2)
                        t1v = t1.rearrange("p (h half i) -> p h half i", h=HL, half=2)
                        t2v = t2.rearrange("p (h half i) -> p h half i", h=HL, half=2)
                        nc.vector.tensor_sub(rv[:, :, 0:1, :], t1v[:, :, 0:1, :],
                                             t2v[:, :, 1:2, :])
                        nc.vector.tensor_add(rv[:, :, 1:2, :], t2v[:, :, 0:1, :],
                                             t1v[:, :, 1:2, :])

                        # transpose the 3 128-col blocks into one PSUM tile
                        pt = ptps.tile([128, 384], BF16, tag="pt", name="pt")
                        for cb2 in range(3):
                            nc.tensor.transpose(
                                pt[:, cb2 * 128:(cb2 + 1) * 128],
                                ro[:, cb2 * 128:(cb2 + 1) * 128], id_sb
                            )
                        if qk == 0:
                            qv = qT_all.rearrange("p (c t) -> p c t", c=3)
                            nc.scalar.copy(qv[:, :, tsl],
                                           pt.rearrange("p (c t) -> p c t", c=3))
                        else:
                            ptv = pt.rearrange("p (c t) -> p c t", c=3)
                            nc.scalar.copy(kTp_v[0:64, :, 0:1, tsl],
                                           ptv[0:64, :, :])
                            nc.scalar.copy(kTp_v[64:128, :, 1:2, tsl],
                                           ptv[64:128, :, :])

            # ================= phase 2: attention =================
            # Flattened (pair, sq, j) work list, software-pipelined: scores
            # for j+1, j+2 are emitted before the exp/PV tail of j so the
            # in-order tensor queue never stalls on the Act-engine exp.
            with (
                tc.tile_pool(name="psS", bufs=3, space="PSUM") as psS,
                tc.tile_pool(name="psO", bufs=2, space="PSUM") as psO,
            ):
                work = []
                for c in range(3):
                    for sq in range(4):
                        njs = 4 * sq + 4
                        for j in range(njs):
                            work.append((c, sq, j, njs))

                LEAD = 2
                state = {}
                po_state = {}

                def emit_scores(idx):
                    c, sq, j, njs = work[idx]
                    hA, hB = 2 * c, 2 * c + 1
                    diag_r = j - 4 * sq
                    col0 = 512 * sq + max(0, diag_r) * 128
                    N = 512 * (sq + 1) - col0
                    ls = col0 - 512 * sq
                    # B head block always at the bank boundary: concurrent
                    # row-tiled matmuls into the SAME PSUM bank crash the
                    # device, so A and B must land in different banks
                    off = 512
                    ps = psS.tile([128, 1024], F32, tag="ps", name="ps")
                    if diag_r >= 0:
                        # causal mask pre-exp: initialize the diag block's
                        # PSUM with -1e5 above the diagonal (zeros elsewhere)
                        # so the DVE never touches the mask and PV does not
                        # wait on a post-exp multiply
                        nc.tensor.matmul(ps[:, 0:N], lhsT=negid_sb,
                                         rhs=triu_sb[:, 0:N],
                                         start=True, stop=False)
                        nc.tensor.matmul(ps[:, off:off + N], lhsT=negid_sb,
                                         rhs=triu_sb[:, 0:N],
                                         start=True, stop=False)
                    st = diag_r < 0
                    # two heads concurrently via PE row tiling:
                    # hA in rows 0:63, hB in rows 64:127
                    nc.tensor.matmul(
                        ps[:, 0:N],
                        lhsT=kTp_all[0:64, hA * T + j * 128:
                                     hA * T + (j + 1) * 128],
                        rhs=qT_all[0:64, c * T + col0:c * T + col0 + N],
                        start=st, stop=True,
                    )
                    nc.tensor.matmul(
                        ps[:, off:off + N],
                        lhsT=kTp_all[64:128, hB * T + j * 128:
                                     hB * T + (j + 1) * 128],
                        rhs=qT_all[64:128, c * T + col0:c * T + col0 + N],
                        start=st, stop=True,
                    )
                    state[idx] = (ps, c, sq, j, njs, N, ls, diag_r, off)

                def emit_tail(idx):
                    ps, c, sq, j, njs, N, ls, diag_r, off = state.pop(idx)
                    hA, hB = 2 * c, 2 * c + 1
                    pP = pPp.tile([128, 1024], BF16, tag="pP", name="pP")
                    # one 3D-AP exp covers both heads' live columns and
                    # skips the dead gap between them on diag blocks
                    psv = ps.rearrange("p (h q) -> p h q", h=2)
                    pPv = pP.rearrange("p (h q) -> p h q", h=2)
                    nc.scalar.activation(pPv[:, :, 0:N], psv[:, :, 0:N],
                                         AF.Exp, scale=0.125)
                    if j == 0:
                        po_state[(c, sq)] = (
                            psO.tile([D + 1, 512], F32, tag="po", name="poA"),
                            psO.tile([D + 1, 512], F32, tag="po", name="poB"),
                        )
                    poA, poB = po_state[(c, sq)]
                    nc.tensor.matmul(
                        poA[:, ls:ls + N],
                        lhsT=v_sb[j][:, hA, 0:D + 1],
                        rhs=pP[:, 0:N],
                        start=(j == 0), stop=(j == njs - 1),
                    )
                    nc.tensor.matmul(
                        poB[:, ls:ls + N],
                        lhsT=v_sb[j][:, hB, 0:D + 1],
                        rhs=pP[:, off:off + N],
                        start=(j == 0), stop=(j == njs - 1),
                    )
                    if j == njs - 1:
                        poA, poB = po_state.pop((c, sq))
                        for (h, po) in ((hA, poA), (hB, poB)):
                            # copy out fast so the po bank frees for the
                            # next sq; normalize from the SBUF copy
                            posb = small.tile([D + 1, 512], F32, tag="posb",
                                              name="posb")
                            nc.vector.tensor_copy(out=posb, in_=po[0:D + 1, :])
                            rden = small.tile([1, 512], F32, tag="rden",
                                              name="rden")
                            nc.vector.tensor_copy(out=rden,
                                                  in_=posb[D:D + 1, :])
                            rec = small.tile([1, 512], F32, tag="rec",
                                             name="rec")
                            nc.vector.reciprocal_approx_fast(rec, rden)
                            bc = small.tile([64, 512], F32, tag="bc", name="bc")
                            nc.gpsimd.partition_broadcast(bc, rec)
                            nc.vector.tensor_mul(
                                yT64[h][:, bass.ts(sq, 512)], posb[0:D, :], bc
                            )
                            nc.sync.dma_start(
                                out=yT128[c][(h % 2) * 64:(h % 2) * 64 + 64,
                                             bass.ts(sq, 512)],
                                in_=yT64[h][:, bass.ts(sq, 512)],
                            )

                for i in range(len(work)):
                    emit_scores(i)
                    if i >= LEAD:
                        emit_tail(i - LEAD)
                for i in range(len(work) - LEAD, len(work)):
                    emit_tail(i)

            # ================= phase 3: output projection (partial) ============
            with (
                tc.tile_pool(name="pjpsum", bufs=4, space="PSUM") as pjps,
                tc.tile_pool(name="pjout", bufs=6) as pjout,
            ):
                for tb in range(TB):
                    tsl = bass.ts(tb, 128)
                    ppA = pjps.tile([128, 512], F32, tag="ppA", name="ppA")
                    ppB = pjps.tile([128, 256], F32, tag="ppB", name="ppB")
                    for fb in range(3):
                        # one stationary load per fb feeds both oc chunks
                        nc.tensor.matmul(
                            ppA,
                            lhsT=yT128[fb][:, tsl],
                            rhs=wp_sb[fb][:, 0:512],
                            start=(fb == 0), stop=(fb == 2),
                        )
                        nc.tensor.matmul(
                            ppB,
                            lhsT=yT128[fb][:, tsl],
                            rhs=wp_sb[fb][:, 512:768],
                            start=(fb == 0), stop=(fb == 2),
                        )
                    for (pp, oc0, ocn, eng) in ((ppA, 0, 512, tb % 2),
                                                (ppB, 512, 256, (tb + 1) % 2)):
                        osb = pjout.tile([128, 512], BF16, tag="osb", name="osb")
                        if eng == 0:
                            nc.scalar.copy(osb[:, 0:ocn], pp[:, 0:ocn])
                        else:
                            nc.vector.tensor_copy(out=osb[:, 0:ocn],
                                                  in_=pp[:, 0:ocn])
                        nc.sync.dma_start(out=out[tsl, oc0:oc0 + ocn],
                                          in_=osb[:, 0:ocn])

    nc.finalize()
    return nc


def _bf16(a):
    return np.ascontiguousarray(np.asarray(a)).astype(ml_dtypes.bfloat16)


# permutation putting rope pairs into contiguous even/odd halves per head
_PERM64 = np.concatenate([np.arange(0, D, 2), np.arange(1, D, 2)])


def _prep_core(c, x, w_qkv, w_proj, sin_rep, cos_rep, negid_m, triu_m,
               ident_m):
    b, hh = c // 2, c % 2
    wq = w_qkv[0 * C + hh * 384: 0 * C + hh * 384 + 384].reshape(HL, D, C)
    wk = w_qkv[1 * C + hh * 384: 1 * C + hh * 384 + 384].reshape(HL, D, C)
    wv = w_qkv[2 * C + hh * 384: 2 * C + hh * 384 + 384]
    wq = wq[:, _PERM64, :].reshape(HL * D, C)
    wk = wk[:, _PERM64, :].reshape(HL * D, C)
    w_local = np.concatenate([wq, wk, wv], 0)       # (1152, 768)
    return {
        "xT": _bf16(x[b].T),
        "wqkvT": _bf16(w_local.T),
        "wpT": _bf16(w_proj[:, hh * 384: hh * 384 + 384].T),
        "sinr": sin_rep,
        "cosr": cos_rep,
        "negid": negid_m,
        "triu512": triu_m,
        "ident": ident_m,
    }


def kernel(x, w_qkv, w_proj, rope_sin, rope_cos, _trace=False):
    global _CACHED_NC
    x = np.asarray(x, dtype=np.float32)
    w_qkv = np.asarray(w_qkv, dtype=np.float32)
    w_proj = np.asarray(w_proj, dtype=np.float32)
    rope_sin = np.asarray(rope_sin, dtype=np.float32)
    rope_cos = np.asarray(rope_cos, dtype=np.float32)

    # (T, 384): per head block [table(32) | table(32)]
    sin_rep = _bf16(np.tile(np.concatenate([rope_sin, rope_sin], 1), (1, HL)))
    cos_rep = _bf16(np.tile(np.concatenate([rope_cos, rope_cos], 1), (1, HL)))
    negid_m = _bf16(np.eye(128) * -1e5)
    # strict upper triangle (key > query -> masked), zero-padded to 512
    triu_m = np.zeros((128, 512), np.float32)
    triu_m[:, 0:128] = np.arange(128)[:, None] > np.arange(128)[None, :]
    triu_m = _bf16(triu_m)
    ident_m = _bf16(np.eye(128))

    in_maps = [_prep_core(c, x, w_qkv, w_proj, sin_rep, cos_rep, negid_m,
                          triu_m, ident_m)
               for c in range(NCORES)]

    if _CACHED_NC is None:
        _CACHED_NC = build_nc()
    nc = _CACHED_NC

    try:
        res = run_bass_kernel_spmd(nc, in_maps, core_ids=list(range(NCORES)),
                                   trace=_trace)
    except Exception:
        # transient NRT_EXEC_UNIT_UNRECOVERABLE has been observed once per
        # ~20 runs in this environment; a single retry always recovered
        res = run_bass_kernel_spmd(nc, in_maps, core_ids=list(range(NCORES)),
                                   trace=_trace)
    parts = [res.results[c]["out"].astype(np.float32) for c in range(NCORES)]
    out = np.stack([parts[2 * b] + parts[2 * b + 1] for b in range(B)], 0)
    if _trace:
        return out.astype(np.float32), res
    return out.astype(np.float32)



# revision 38
# speedup vs baseline: 1.0993x; 1.0993x over previous
"""Causal self-attention with RoPE, sharded over 8 TRN2 NeuronCores.

Sharding: data-parallel over B (4 ways) x tensor-parallel over heads
(2 ways, 6 heads each). Each core computes qkv projection, RoPE,
causal attention and a partial output projection for its (batch,
head-half); the host sums the two head-half partials per batch.

Attention processes heads in PAIRS using PE-array row tiling: the
score matmuls contract over d=64, so head 2c runs in PE rows 0:63
and head 2c+1 in rows 64:127 concurrently (tile_position derived
from base partitions) - 2x effective score throughput vs the
zero-padded K=128 form. The causal mask is applied post-exp as a
DVE multiply with a 0/1 lower-triangular matrix on the diagonal
128x128 blocks (no mask matmuls). V carries a ones column (M=65
stationary) so PV also yields the softmax denominator.

RoPE pairs are host-permuted into contiguous even/odd halves per head
(legal because the QK contraction is invariant to permuting head dims
as long as q and k share the permutation, and V is not roped).
"""

import numpy as np
import ml_dtypes

import concourse.bass as bass
import concourse.tile as tile
import concourse.mybir as mybir
from concourse import bacc
from concourse.bass_utils import run_bass_kernel_spmd

B, T, C, H, D = 4, 2048, 768, 12, 64
HL = H // 2          # heads per core
TB = T // 128        # 16 t-blocks
CB = C // 128        # 6 contraction blocks
NCORES = 8

F32 = mybir.dt.float32
BF16 = mybir.dt.bfloat16
AF = mybir.ActivationFunctionType

_CACHED_NC = None


def build_nc():
    nc = bacc.Bacc("TRN2", target_bir_lowering=False)

    xT = nc.declare_dram_parameter("xT", [C, T], BF16, isOutput=False)
    wqkvT = nc.declare_dram_parameter("wqkvT", [C, 3 * HL * D], BF16, isOutput=False)
    wpT = nc.declare_dram_parameter("wpT", [HL * D, C], BF16, isOutput=False)
    sinr = nc.declare_dram_parameter("sinr", [T, HL * D], BF16, isOutput=False)
    cosr = nc.declare_dram_parameter("cosr", [T, HL * D], BF16, isOutput=False)
    negid = nc.declare_dram_parameter("negid", [128, 128], BF16, isOutput=False)
    triu512 = nc.declare_dram_parameter("triu512", [128, 512], BF16, isOutput=False)
    ident = nc.declare_dram_parameter("ident", [128, 128], BF16, isOutput=False)
    out = nc.declare_dram_parameter("out", [T, C], BF16, isOutput=True)

    with tile.TileContext(nc) as tc:
        with (
            tc.tile_pool(name="persist", bufs=1) as persist,
            tc.tile_pool(name="pPp", bufs=10) as pPp,
            tc.tile_pool(name="small", bufs=4) as small,
            tc.tile_pool(name="pjout", bufs=2) as pjout,
        ):
            # ---- persistent SBUF tensors ----
            # qT_all: head-pair block hb occupies cols [hb*2048, (hb+1)*2048);
            # head h lives at rows (h%2)*64 of block h//2.
            qT_all = persist.tile([128, 3 * T], BF16, tag="qT", name="qT_all")
            # kTp_all: head h occupies cols [h*2048, ...), rows (h%2)*64.
            # The other 64 rows of each col block are never read (K=64
            # row-tiled score matmuls).
            kTp_all = persist.tile([128, HL * T], BF16, tag="kT", name="kTp_all")
            # v tiles: per t-block, head h at cols [h*128, h*128+64] = v,
            # col h*128+64 = ones; cols beyond 65 are never read (M=65).
            v_sb = [persist.tile([128, HL, 128], BF16, tag=f"v{i}", name=f"v{i}")
                    for i in range(TB)]
            wp_sb = [persist.tile([128, C], BF16, tag=f"wp{i}", name=f"wp{i}")
                     for i in range(3)]
            yT64 = [persist.tile([64, T], BF16, tag=f"y64_{i}", name=f"y64_{i}")
                    for i in range(HL)]
            yT128 = [persist.tile([128, T], BF16, tag=f"y128_{i}", name=f"y128_{i}")
                     for i in range(3)]
            negid_sb = persist.tile([128, 128], BF16, tag="negid")
            triu_sb = persist.tile([128, 512], BF16, tag="triu512")
            id_sb = persist.tile([128, 128], BF16, tag="ident")

            nc.gpsimd.dma_start(out=negid_sb, in_=negid[:, :])
            nc.gpsimd.dma_start(out=triu_sb, in_=triu512[:, :])
            nc.gpsimd.dma_start(out=id_sb, in_=ident[:, :])
            for i in range(3):
                nc.gpsimd.dma_start(out=wp_sb[i], in_=wpT[i * 128:(i + 1) * 128, :])

            kTp_v = kTp_all.rearrange("p (c two t) -> p c two t", c=3, two=2)

            # ================= phase 1: qkv + rope + transpose =================
            with (
                tc.tile_pool(name="p1in", bufs=1) as p1in,
                tc.tile_pool(name="p1work", bufs=3) as p1w,
                tc.tile_pool(name="p1psum", bufs=6, space="PSUM") as p1ps,
                tc.tile_pool(name="ptpsum", bufs=2, space="PSUM") as ptps,
            ):
                xT_sb = [p1in.tile([128, T], BF16, tag=f"xT{i}", name=f"xTs{i}")
                         for i in range(CB)]
                w_sb = [p1in.tile([128, 3 * HL * D], BF16, tag=f"w{i}", name=f"ws{i}")
                        for i in range(CB)]
                for i in range(CB):
                    # stage the first t-blocks / q-chunk first so tb=0's qkv
                    # matmuls start while the bulk still streams; x on the
                    # act issue queue, w on sync, so staging issues in
                    # parallel (act is idle this early)
                    nc.scalar.dma_start(out=xT_sb[i][:, 0:512],
                                        in_=xT[i * 128:(i + 1) * 128, 0:512])
                    nc.sync.dma_start(out=w_sb[i][:, 0:384],
                                      in_=wqkvT[i * 128:(i + 1) * 128, 0:384])
                for i in range(CB):
                    nc.scalar.dma_start(out=xT_sb[i][:, 512:T],
                                        in_=xT[i * 128:(i + 1) * 128, 512:T])
                    nc.sync.dma_start(out=w_sb[i][:, 384:],
                                      in_=wqkvT[i * 128:(i + 1) * 128, 384:])

                for tb in range(TB):
                    if tb == 1:
                        # one-time ones column for the denominator trick,
                        # emitted here so it overlaps phase 1 instead of
                        # gating tb=0
                        for i in range(TB):
                            nc.gpsimd.memset(v_sb[i][:, :, D:D + 1], 1.0)
                    tsl = bass.ts(tb, 128)
                    ps_qkv = []
                    for chunk in range(3):
                        ps = p1ps.tile([128, 384], F32, tag="qkv", name="psqkv")
                        for cb in range(CB):
                            nc.tensor.matmul(
                                ps,
                                lhsT=xT_sb[cb][:, tsl],
                                rhs=w_sb[cb][:, chunk * 384:(chunk + 1) * 384],
                                start=(cb == 0),
                                stop=(cb == CB - 1),
                            )
                        ps_qkv.append(ps)

                    sin_sb = p1w.tile([128, HL * D], BF16, tag="sin", name="sin_sb")
                    cos_sb = p1w.tile([128, HL * D], BF16, tag="cos", name="cos_sb")
                    nc.gpsimd.dma_start(out=sin_sb, in_=sinr[tsl, :])
                    nc.gpsimd.dma_start(out=cos_sb, in_=cosr[tsl, :])

                    # v: pack t-major (ones col already set)
                    nc.scalar.copy(
                        v_sb[tb][:, :, 0:D],
                        ps_qkv[2].rearrange("p (h d) -> p h d", h=HL),
                    )

                    # rope directly from PSUM; halves layout (host-permuted):
                    # per head cols [evens(32) | odds(32)]
                    for qk in range(2):
                        ro = p1w.tile([128, HL * D], BF16, tag=f"ro{qk}",
                                      name=f"ro{qk}")
                        t1 = p1w.tile([128, HL * D], BF16, tag="t1", name="t1")
                        t2 = p1w.tile([128, HL * D], BF16, tag="t2", name="t2")
                        nc.vector.tensor_mul(t1, ps_qkv[qk], cos_sb)
                        nc.vector.tensor_mul(t2, ps_qkv[qk], sin_sb)
                        rv = ro.rearrange("p (h half i) -> p h half i", h=HL, half=2)
                        t1v = t1.rearrange("p (h half i) -> p h half i", h=HL, half=2)
                        t2v = t2.rearrange("p (h half i) -> p h half i", h=HL, half=2)
                        nc.vector.tensor_sub(rv[:, :, 0:1, :], t1v[:, :, 0:1, :],
                                             t2v[:, :, 1:2, :])
                        nc.vector.tensor_add(rv[:, :, 1:2, :], t2v[:, :, 0:1, :],
                                             t1v[:, :, 1:2, :])

                        # transpose the 3 128-col blocks into one PSUM tile
                        pt = ptps.tile([128, 384], BF16, tag="pt", name="pt")
                        for cb2 in range(3):
                            nc.tensor.transpose(
                                pt[:, cb2 * 128:(cb2 + 1) * 128],
                                ro[:, cb2 * 128:(cb2 + 1) * 128], id_sb
                            )
                        if qk == 0:
                            qv = qT_all.rearrange("p (c t) -> p c t", c=3)
                            nc.scalar.copy(qv[:, :, tsl],
                                           pt.rearrange("p (c t) -> p c t", c=3))
                        else:
                            ptv = pt.rearrange("p (c t) -> p c t", c=3)
                            nc.scalar.copy(kTp_v[0:64, :, 0:1, tsl],
                                           ptv[0:64, :, :])
                            nc.scalar.copy(kTp_v[64:128, :, 1:2, tsl],
                                           ptv[64:128, :, :])

            # ================= phase 2: attention =================
            # Flattened (pair, sq, j) work list, software-pipelined: scores
            # for j+1, j+2 are emitted before the exp/PV tail of j so the
            # in-order tensor queue never stalls on the Act-engine exp.
            with (
                tc.tile_pool(name="psS", bufs=3, space="PSUM") as psS,
                tc.tile_pool(name="psO", bufs=2, space="PSUM") as psO,
            ):
                work = []
                for c in range(3):
                    for sq in range(4):
                        njs = 4 * sq + 4
                        for j in range(njs):
                            work.append((c, sq, j, njs))

                LEAD = 2
                state = {}
                po_state = {}

                def emit_scores(idx):
                    c, sq, j, njs = work[idx]
                    hA, hB = 2 * c, 2 * c + 1
                    diag_r = j - 4 * sq
                    col0 = 512 * sq + max(0, diag_r) * 128
                    N = 512 * (sq + 1) - col0
                    ls = col0 - 512 * sq
                    # B head block always at the bank boundary: concurrent
                    # row-tiled matmuls into the SAME PSUM bank crash the
                    # device, so A and B must land in different banks
                    off = 512
                    ps = psS.tile([128, 1024], F32, tag="ps", name="ps")
                    if diag_r >= 0:
                        # causal mask pre-exp: initialize the diag block's
                        # PSUM with -1e5 above the diagonal (zeros elsewhere)
                        # so the DVE never touches the mask and PV does not
                        # wait on a post-exp multiply
                        nc.tensor.matmul(ps[:, 0:N], lhsT=negid_sb,
                                         rhs=triu_sb[:, 0:N],
                                         start=True, stop=False)
                        nc.tensor.matmul(ps[:, off:off + N], lhsT=negid_sb,
                                         rhs=triu_sb[:, 0:N],
                                         start=True, stop=False)
                    st = diag_r < 0
                    # two heads concurrently via PE row tiling:
                    # hA in rows 0:63, hB in rows 64:127
                    nc.tensor.matmul(
                        ps[:, 0:N],
                        lhsT=kTp_all[0:64, hA * T + j * 128:
                                     hA * T + (j + 1) * 128],
                        rhs=qT_all[0:64, c * T + col0:c * T + col0 + N],
                        start=st, stop=True,
                    )
                    nc.tensor.matmul(
                        ps[:, off:off + N],
                        lhsT=kTp_all[64:128, hB * T + j * 128:
                                     hB * T + (j + 1) * 128],
                        rhs=qT_all[64:128, c * T + col0:c * T + col0 + N],
                        start=st, stop=True,
                    )
                    state[idx] = (ps, c, sq, j, njs, N, ls, diag_r, off)

                def emit_tail(idx):
                    ps, c, sq, j, njs, N, ls, diag_r, off = state.pop(idx)
                    hA, hB = 2 * c, 2 * c + 1
                    pP = pPp.tile([128, 1024], BF16, tag="pP", name="pP")
                    # one 3D-AP exp covers both heads' live columns and
                    # skips the dead gap between them on diag blocks
                    psv = ps.rearrange("p (h q) -> p h q", h=2)
                    pPv = pP.rearrange("p (h q) -> p h q", h=2)
                    nc.scalar.activation(pPv[:, :, 0:N], psv[:, :, 0:N],
                                         AF.Exp, scale=0.125)
                    if j == 0:
                        po_state[(c, sq)] = (
                            psO.tile([128, 512], F32, tag="po", name="poA"),
                            psO.tile([128, 512], F32, tag="po", name="poB"),
                        )
                    poA, poB = po_state[(c, sq)]
                    nc.tensor.matmul(
                        poA[0:D + 1, ls:ls + N],
                        lhsT=v_sb[j][:, hA, 0:D + 1],
                        rhs=pP[:, 0:N],
                        start=(j == 0), stop=(j == njs - 1),
                    )
                    nc.tensor.matmul(
                        poB[0:D + 1, ls:ls + N],
                        lhsT=v_sb[j][:, hB, 0:D + 1],
                        rhs=pP[:, off:off + N],
                        start=(j == 0), stop=(j == njs - 1),
                    )
                    if j == njs - 1:
                        poA, poB = po_state.pop((c, sq))
                        for (h, po) in ((hA, poA), (hB, poB)):
                            # copy out fast so the po bank frees for the
                            # next sq; normalize from the SBUF copy
                            posb = small.tile([D + 1, 512], F32, tag="posb",
                                              name="posb")
                            nc.vector.tensor_copy(out=posb, in_=po[0:D + 1, :])
                            rden = small.tile([1, 512], F32, tag="rden",
                                              name="rden")
                            nc.vector.tensor_copy(out=rden,
                                                  in_=posb[D:D + 1, :])
                            rec = small.tile([1, 512], F32, tag="rec",
                                             name="rec")
                            nc.vector.reciprocal_approx_fast(rec, rden)
                            bc = small.tile([64, 512], F32, tag="bc", name="bc")
                            nc.gpsimd.partition_broadcast(bc, rec)
                            nc.vector.tensor_mul(
                                yT64[h][:, bass.ts(sq, 512)], posb[0:D, :], bc
                            )
                            nc.sync.dma_start(
                                out=yT128[c][(h % 2) * 64:(h % 2) * 64 + 64,
                                             bass.ts(sq, 512)],
                                in_=yT64[h][:, bass.ts(sq, 512)],
                            )

                def proj_unit(tb):
                    # output projection for one t-block, ring-sharing the po
                    # PSUM pool; runs inside phase 2 once the last pair's
                    # q-chunk is normalized (PE has slack; act is the pacer)
                    tsl = bass.ts(tb, 128)
                    ppA = psO.tile([128, 512], F32, tag="po", name="ppA")
                    ppB = psO.tile([128, 512], F32, tag="po", name="ppB")
                    for fb in range(3):
                        nc.tensor.matmul(
                            ppA,
                            lhsT=yT128[fb][:, tsl],
                            rhs=wp_sb[fb][:, 0:512],
                            start=(fb == 0), stop=(fb == 2),
                        )
                        nc.tensor.matmul(
                            ppB[:, 0:256],
                            lhsT=yT128[fb][:, tsl],
                            rhs=wp_sb[fb][:, 512:768],
                            start=(fb == 0), stop=(fb == 2),
                        )
                    for (pp, oc0, ocn) in ((ppA, 0, 512), (ppB, 512, 256)):
                        osb = pjout.tile([128, 512], BF16, tag="osb",
                                         name="osb")
                        nc.vector.tensor_copy(out=osb[:, 0:ocn],
                                              in_=pp[:, 0:ocn])
                        nc.sync.dma_start(out=out[tsl, oc0:oc0 + ocn],
                                          in_=osb[:, 0:ocn])

                projq = []

                def tail_and_proj(i):
                    emit_tail(i)
                    c, sq, j, njs = work[i]
                    if c == 2 and j == njs - 1:
                        projq.extend(range(4 * sq, 4 * sq + 4))
                    if projq:
                        proj_unit(projq.pop(0))

                for i in range(len(work)):
                    emit_scores(i)
                    if i >= LEAD:
                        tail_and_proj(i - LEAD)
                for i in range(len(work) - LEAD, len(work)):
                    tail_and_proj(i)
                for tb in projq:
                    proj_unit(tb)

    nc.finalize()
    return nc


def _bf16(a):
    return np.ascontiguousarray(np.asarray(a)).astype(ml_dtypes.bfloat16)


# permutation putting rope pairs into contiguous even/odd halves per head
_PERM64 = np.concatenate([np.arange(0, D, 2), np.arange(1, D, 2)])


def _prep_core(c, x, w_qkv, w_proj, sin_rep, cos_rep, negid_m, triu_m,
               ident_m):
    b, hh = c // 2, c % 2
    wq = w_qkv[0 * C + hh * 384: 0 * C + hh * 384 + 384].reshape(HL, D, C)
    wk = w_qkv[1 * C + hh * 384: 1 * C + hh * 384 + 384].reshape(HL, D, C)
    wv = w_qkv[2 * C + hh * 384: 2 * C + hh * 384 + 384]
    wq = wq[:, _PERM64, :].reshape(HL * D, C)
    wk = wk[:, _PERM64, :].reshape(HL * D, C)
    w_local = np.concatenate([wq, wk, wv], 0)       # (1152, 768)
    return {
        "xT": _bf16(x[b].T),
        "wqkvT": _bf16(w_local.T),
        "wpT": _bf16(w_proj[:, hh * 384: hh * 384 + 384].T),
        "sinr": sin_rep,
        "cosr": cos_rep,
        "negid": negid_m,
        "triu512": triu_m,
        "ident": ident_m,
    }


def kernel(x, w_qkv, w_proj, rope_sin, rope_cos, _trace=False):
    global _CACHED_NC
    x = np.asarray(x, dtype=np.float32)
    w_qkv = np.asarray(w_qkv, dtype=np.float32)
    w_proj = np.asarray(w_proj, dtype=np.float32)
    rope_sin = np.asarray(rope_sin, dtype=np.float32)
    rope_cos = np.asarray(rope_cos, dtype=np.float32)

    # (T, 384): per head block [table(32) | table(32)]
    sin_rep = _bf16(np.tile(np.concatenate([rope_sin, rope_sin], 1), (1, HL)))
    cos_rep = _bf16(np.tile(np.concatenate([rope_cos, rope_cos], 1), (1, HL)))
    negid_m = _bf16(np.eye(128) * -1e5)
    # strict upper triangle (key > query -> masked), zero-padded to 512
    triu_m = np.zeros((128, 512), np.float32)
    triu_m[:, 0:128] = np.arange(128)[:, None] > np.arange(128)[None, :]
    triu_m = _bf16(triu_m)
    ident_m = _bf16(np.eye(128))

    in_maps = [_prep_core(c, x, w_qkv, w_proj, sin_rep, cos_rep, negid_m,
                          triu_m, ident_m)
               for c in range(NCORES)]

    if _CACHED_NC is None:
        _CACHED_NC = build_nc()
    nc = _CACHED_NC

    try:
        res = run_bass_kernel_spmd(nc, in_maps, core_ids=list(range(NCORES)),
                                   trace=_trace)
    except Exception:
        # transient NRT_EXEC_UNIT_UNRECOVERABLE has been observed once per
        # ~20 runs in this environment; a single retry always recovered
        res = run_bass_kernel_spmd(nc, in_maps, core_ids=list(range(NCORES)),
                                   trace=_trace)
    parts = [res.results[c]["out"].astype(np.float32) for c in range(NCORES)]
    out = np.stack([parts[2 * b] + parts[2 * b + 1] for b in range(B)], 0)
    if _trace:
        return out.astype(np.float32), res
    return out.astype(np.float32)



# revision 39
# speedup vs baseline: 1.1842x; 1.0773x over previous
"""Causal self-attention with RoPE, sharded over 8 TRN2 NeuronCores.

Sharding: data-parallel over B (4 ways) x tensor-parallel over heads
(2 ways, 6 heads each). Each core computes qkv projection, RoPE,
causal attention and a partial output projection for its (batch,
head-half); the host sums the two head-half partials per batch.

Attention processes heads in PAIRS using PE-array row tiling: the
score matmuls contract over d=64, so head 2c runs in PE rows 0:63
and head 2c+1 in rows 64:127 concurrently (tile_position derived
from base partitions) - 2x effective score throughput vs the
zero-padded K=128 form. The causal mask is applied post-exp as a
DVE multiply with a 0/1 lower-triangular matrix on the diagonal
128x128 blocks (no mask matmuls). V carries a ones column (M=65
stationary) so PV also yields the softmax denominator.

RoPE pairs are host-permuted into contiguous even/odd halves per head
(legal because the QK contraction is invariant to permuting head dims
as long as q and k share the permutation, and V is not roped).
"""

import numpy as np
import ml_dtypes

import concourse.bass as bass
import concourse.tile as tile
import concourse.mybir as mybir
from concourse import bacc
from concourse.bass_utils import run_bass_kernel_spmd

B, T, C, H, D = 4, 2048, 768, 12, 64
HL = H // 2          # heads per core
TB = T // 128        # 16 t-blocks
CB = C // 128        # 6 contraction blocks
NCORES = 8

F32 = mybir.dt.float32
BF16 = mybir.dt.bfloat16
AF = mybir.ActivationFunctionType

_CACHED_NC = None


def build_nc():
    nc = bacc.Bacc("TRN2", target_bir_lowering=False)

    xT = nc.declare_dram_parameter("xT", [C, T], BF16, isOutput=False)
    wqkvT = nc.declare_dram_parameter("wqkvT", [C, 3 * HL * D], BF16, isOutput=False)
    wpT = nc.declare_dram_parameter("wpT", [HL * D, C], BF16, isOutput=False)
    sinr = nc.declare_dram_parameter("sinr", [T, HL * D], BF16, isOutput=False)
    cosr = nc.declare_dram_parameter("cosr", [T, HL * D], BF16, isOutput=False)
    negid = nc.declare_dram_parameter("negid", [128, 128], BF16, isOutput=False)
    triu512 = nc.declare_dram_parameter("triu512", [128, 512], BF16, isOutput=False)
    ident = nc.declare_dram_parameter("ident", [128, 128], BF16, isOutput=False)
    out = nc.declare_dram_parameter("out", [T, C], BF16, isOutput=True)

    with tile.TileContext(nc) as tc:
        with (
            tc.tile_pool(name="persist", bufs=1) as persist,
            tc.tile_pool(name="pPp", bufs=10) as pPp,
            tc.tile_pool(name="small", bufs=4) as small,
        ):
            # ---- persistent SBUF tensors ----
            # qT_all: head-pair block hb occupies cols [hb*2048, (hb+1)*2048);
            # head h lives at rows (h%2)*64 of block h//2.
            qT_all = persist.tile([128, 3 * T], BF16, tag="qT", name="qT_all")
            # kTp_all: head h occupies cols [h*2048, ...), rows (h%2)*64.
            # The other 64 rows of each col block are never read (K=64
            # row-tiled score matmuls).
            kTp_all = persist.tile([128, HL * T], BF16, tag="kT", name="kTp_all")
            # v tiles: per t-block, head h at cols [h*128, h*128+64] = v,
            # col h*128+64 = ones; cols beyond 65 are never read (M=65).
            v_sb = [persist.tile([128, HL, 128], BF16, tag=f"v{i}", name=f"v{i}")
                    for i in range(TB)]
            wp_sb = [persist.tile([128, C], BF16, tag=f"wp{i}", name=f"wp{i}")
                     for i in range(3)]
            yT64 = [persist.tile([64, T], BF16, tag=f"y64_{i}", name=f"y64_{i}")
                    for i in range(HL)]
            yT128 = [persist.tile([128, T], BF16, tag=f"y128_{i}", name=f"y128_{i}")
                     for i in range(3)]
            negid_sb = persist.tile([128, 128], BF16, tag="negid")
            triu_sb = persist.tile([128, 512], BF16, tag="triu512")
            id_sb = persist.tile([128, 128], BF16, tag="ident")

            nc.gpsimd.dma_start(out=negid_sb, in_=negid[:, :])
            nc.gpsimd.dma_start(out=triu_sb, in_=triu512[:, :])
            nc.gpsimd.dma_start(out=id_sb, in_=ident[:, :])
            for i in range(3):
                nc.gpsimd.dma_start(out=wp_sb[i], in_=wpT[i * 128:(i + 1) * 128, :])

            kTp_v = kTp_all.rearrange("p (c two t) -> p c two t", c=3, two=2)

            # ================= phase 1: qkv + rope + transpose =================
            with (
                tc.tile_pool(name="p1in", bufs=1) as p1in,
                tc.tile_pool(name="p1work", bufs=3) as p1w,
                tc.tile_pool(name="p1psum", bufs=6, space="PSUM") as p1ps,
                tc.tile_pool(name="ptpsum", bufs=2, space="PSUM") as ptps,
            ):
                xT_sb = [p1in.tile([128, T], BF16, tag=f"xT{i}", name=f"xTs{i}")
                         for i in range(CB)]
                w_sb = [p1in.tile([128, 3 * HL * D], BF16, tag=f"w{i}", name=f"ws{i}")
                        for i in range(CB)]
                for i in range(CB):
                    # stage the first t-blocks / q-chunk first so tb=0's qkv
                    # matmuls start while the bulk still streams; x on the
                    # act issue queue, w on sync, so staging issues in
                    # parallel (act is idle this early)
                    nc.scalar.dma_start(out=xT_sb[i][:, 0:512],
                                        in_=xT[i * 128:(i + 1) * 128, 0:512])
                    nc.sync.dma_start(out=w_sb[i][:, 0:384],
                                      in_=wqkvT[i * 128:(i + 1) * 128, 0:384])
                for i in range(CB):
                    nc.scalar.dma_start(out=xT_sb[i][:, 512:T],
                                        in_=xT[i * 128:(i + 1) * 128, 512:T])
                    nc.sync.dma_start(out=w_sb[i][:, 384:],
                                      in_=wqkvT[i * 128:(i + 1) * 128, 384:])

                for tb in range(TB):
                    if tb == 1:
                        # one-time ones column for the denominator trick,
                        # emitted here so it overlaps phase 1 instead of
                        # gating tb=0
                        for i in range(TB):
                            nc.gpsimd.memset(v_sb[i][:, :, D:D + 1], 1.0)
                    tsl = bass.ts(tb, 128)
                    ps_qkv = []
                    for chunk in range(3):
                        ps = p1ps.tile([128, 384], F32, tag="qkv", name="psqkv")
                        for cb in range(CB):
                            nc.tensor.matmul(
                                ps,
                                lhsT=xT_sb[cb][:, tsl],
                                rhs=w_sb[cb][:, chunk * 384:(chunk + 1) * 384],
                                start=(cb == 0),
                                stop=(cb == CB - 1),
                            )
                        ps_qkv.append(ps)

                    sin_sb = p1w.tile([128, HL * D], BF16, tag="sin", name="sin_sb")
                    cos_sb = p1w.tile([128, HL * D], BF16, tag="cos", name="cos_sb")
                    nc.gpsimd.dma_start(out=sin_sb, in_=sinr[tsl, :])
                    nc.gpsimd.dma_start(out=cos_sb, in_=cosr[tsl, :])

                    # v: pack t-major (ones col already set)
                    nc.scalar.copy(
                        v_sb[tb][:, :, 0:D],
                        ps_qkv[2].rearrange("p (h d) -> p h d", h=HL),
                    )

                    # rope directly from PSUM; halves layout (host-permuted):
                    # per head cols [evens(32) | odds(32)]
                    for qk in range(2):
                        ro = p1w.tile([128, HL * D], BF16, tag=f"ro{qk}",
                                      name=f"ro{qk}")
                        t1 = p1w.tile([128, HL * D], BF16, tag="t1", name="t1")
                        t2 = p1w.tile([128, HL * D], BF16, tag="t2", name="t2")
                        nc.vector.tensor_mul(t1, ps_qkv[qk], cos_sb)
                        nc.vector.tensor_mul(t2, ps_qkv[qk], sin_sb)
                        rv = ro.rearrange("p (h half i) -> p h half i", h=HL, half=2)
                        t1v = t1.rearrange("p (h half i) -> p h half i", h=HL, half=2)
                        t2v = t2.rearrange("p (h half i) -> p h half i", h=HL, half=2)
                        nc.vector.tensor_sub(rv[:, :, 0:1, :], t1v[:, :, 0:1, :],
                                             t2v[:, :, 1:2, :])
                        nc.vector.tensor_add(rv[:, :, 1:2, :], t2v[:, :, 0:1, :],
                                             t1v[:, :, 1:2, :])

                        # transpose the 3 128-col blocks into one PSUM tile
                        pt = ptps.tile([128, 384], BF16, tag="pt", name="pt")
                        for cb2 in range(3):
                            nc.tensor.transpose(
                                pt[:, cb2 * 128:(cb2 + 1) * 128],
                                ro[:, cb2 * 128:(cb2 + 1) * 128], id_sb
                            )
                        if qk == 0:
                            qv = qT_all.rearrange("p (c t) -> p c t", c=3)
                            nc.scalar.copy(qv[:, :, tsl],
                                           pt.rearrange("p (c t) -> p c t", c=3))
                        else:
                            ptv = pt.rearrange("p (c t) -> p c t", c=3)
                            nc.scalar.copy(kTp_v[0:64, :, 0:1, tsl],
                                           ptv[0:64, :, :])
                            nc.scalar.copy(kTp_v[64:128, :, 1:2, tsl],
                                           ptv[64:128, :, :])

            # ================= phase 2: attention =================
            # Flattened (pair, sq, j) work list, software-pipelined: scores
            # for j+1, j+2 are emitted before the exp/PV tail of j so the
            # in-order tensor queue never stalls on the Act-engine exp.
            with (
                tc.tile_pool(name="psS", bufs=3, space="PSUM") as psS,
                tc.tile_pool(name="psO", bufs=2, space="PSUM") as psO,
            ):
                work = []
                for c in range(3):
                    for sq in range(4):
                        njs = 4 * sq + 4
                        for j in range(njs):
                            work.append((c, sq, j, njs))

                LEAD = 2
                state = {}
                po_state = {}

                def emit_scores(idx):
                    c, sq, j, njs = work[idx]
                    hA, hB = 2 * c, 2 * c + 1
                    diag_r = j - 4 * sq
                    col0 = 512 * sq + max(0, diag_r) * 128
                    N = 512 * (sq + 1) - col0
                    ls = col0 - 512 * sq
                    # B head block always at the bank boundary: concurrent
                    # row-tiled matmuls into the SAME PSUM bank crash the
                    # device, so A and B must land in different banks
                    off = 512
                    ps = psS.tile([128, 1024], F32, tag="ps", name="ps")
                    if diag_r >= 0:
                        # causal mask pre-exp: initialize the diag block's
                        # PSUM with -1e5 above the diagonal (zeros elsewhere)
                        # so the DVE never touches the mask and PV does not
                        # wait on a post-exp multiply
                        nc.tensor.matmul(ps[:, 0:N], lhsT=negid_sb,
                                         rhs=triu_sb[:, 0:N],
                                         start=True, stop=False)
                        nc.tensor.matmul(ps[:, off:off + N], lhsT=negid_sb,
                                         rhs=triu_sb[:, 0:N],
                                         start=True, stop=False)
                    st = diag_r < 0
                    # two heads concurrently via PE row tiling:
                    # hA in rows 0:63, hB in rows 64:127
                    nc.tensor.matmul(
                        ps[:, 0:N],
                        lhsT=kTp_all[0:64, hA * T + j * 128:
                                     hA * T + (j + 1) * 128],
                        rhs=qT_all[0:64, c * T + col0:c * T + col0 + N],
                        start=st, stop=True,
                    )
                    nc.tensor.matmul(
                        ps[:, off:off + N],
                        lhsT=kTp_all[64:128, hB * T + j * 128:
                                     hB * T + (j + 1) * 128],
                        rhs=qT_all[64:128, c * T + col0:c * T + col0 + N],
                        start=st, stop=True,
                    )
                    state[idx] = (ps, c, sq, j, njs, N, ls, diag_r, off)

                def emit_tail(idx):
                    ps, c, sq, j, njs, N, ls, diag_r, off = state.pop(idx)
                    hA, hB = 2 * c, 2 * c + 1
                    pP = pPp.tile([128, 1024], BF16, tag="pP", name="pP")
                    # one 3D-AP exp covers both heads' live columns and
                    # skips the dead gap between them on diag blocks
                    psv = ps.rearrange("p (h q) -> p h q", h=2)
                    pPv = pP.rearrange("p (h q) -> p h q", h=2)
                    nc.scalar.activation(pPv[:, :, 0:N], psv[:, :, 0:N],
                                         AF.Exp, scale=0.125)
                    if j == 0:
                        po_state[(c, sq)] = (
                            psO.tile([D + 1, 512], F32, tag="po", name="poA"),
                            psO.tile([D + 1, 512], F32, tag="po", name="poB"),
                        )
                    poA, poB = po_state[(c, sq)]
                    nc.tensor.matmul(
                        poA[:, ls:ls + N],
                        lhsT=v_sb[j][:, hA, 0:D + 1],
                        rhs=pP[:, 0:N],
                        start=(j == 0), stop=(j == njs - 1),
                    )
                    nc.tensor.matmul(
                        poB[:, ls:ls + N],
                        lhsT=v_sb[j][:, hB, 0:D + 1],
                        rhs=pP[:, off:off + N],
                        start=(j == 0), stop=(j == njs - 1),
                    )
                    if j == njs - 1:
                        poA, poB = po_state.pop((c, sq))
                        for (h, po) in ((hA, poA), (hB, poB)):
                            # copy out fast so the po bank frees for the
                            # next sq; normalize from the SBUF copy
                            posb = small.tile([D + 1, 512], F32, tag="posb",
                                              name="posb")
                            nc.vector.tensor_copy(out=posb, in_=po[0:D + 1, :])
                            rden = small.tile([1, 512], F32, tag="rden",
                                              name="rden")
                            nc.vector.tensor_copy(out=rden,
                                                  in_=posb[D:D + 1, :])
                            rec = small.tile([1, 512], F32, tag="rec",
                                             name="rec")
                            nc.vector.reciprocal_approx_fast(rec, rden)
                            bc = small.tile([64, 512], F32, tag="bc", name="bc")
                            nc.gpsimd.partition_broadcast(bc, rec)
                            nc.vector.tensor_mul(
                                yT64[h][:, bass.ts(sq, 512)], posb[0:D, :], bc
                            )
                            nc.sync.dma_start(
                                out=yT128[c][(h % 2) * 64:(h % 2) * 64 + 64,
                                             bass.ts(sq, 512)],
                                in_=yT64[h][:, bass.ts(sq, 512)],
                            )

                for i in range(len(work)):
                    emit_scores(i)
                    if i >= LEAD:
                        emit_tail(i - LEAD)
                for i in range(len(work) - LEAD, len(work)):
                    emit_tail(i)

            # ================= phase 3: output projection (partial) ============
            with (
                tc.tile_pool(name="pjpsum", bufs=4, space="PSUM") as pjps,
                tc.tile_pool(name="pjout", bufs=6) as pjout,
            ):
                for tb in range(TB):
                    tsl = bass.ts(tb, 128)
                    ppA = pjps.tile([128, 512], F32, tag="ppA", name="ppA")
                    ppB = pjps.tile([128, 256], F32, tag="ppB", name="ppB")
                    for fb in range(3):
                        # one stationary load per fb feeds both oc chunks
                        nc.tensor.matmul(
                            ppA,
                            lhsT=yT128[fb][:, tsl],
                            rhs=wp_sb[fb][:, 0:512],
                            start=(fb == 0), stop=(fb == 2),
                        )
                        nc.tensor.matmul(
                            ppB,
                            lhsT=yT128[fb][:, tsl],
                            rhs=wp_sb[fb][:, 512:768],
                            start=(fb == 0), stop=(fb == 2),
                        )
                    for (pp, oc0, ocn, eng) in ((ppA, 0, 512, tb % 2),
                                                (ppB, 512, 256, (tb + 1) % 2)):
                        osb = pjout.tile([128, 512], BF16, tag="osb", name="osb")
                        if eng == 0:
                            nc.scalar.copy(osb[:, 0:ocn], pp[:, 0:ocn])
                        else:
                            nc.vector.tensor_copy(out=osb[:, 0:ocn],
                                                  in_=pp[:, 0:ocn])
                        nc.sync.dma_start(out=out[tsl, oc0:oc0 + ocn],
                                          in_=osb[:, 0:ocn])

    nc.finalize()
    return nc


def _bf16(a):
    return np.ascontiguousarray(np.asarray(a)).astype(ml_dtypes.bfloat16)


# permutation putting rope pairs into contiguous even/odd halves per head
_PERM64 = np.concatenate([np.arange(0, D, 2), np.arange(1, D, 2)])


def _prep_core(c, x, w_qkv, w_proj, sin_rep, cos_rep, negid_m, triu_m,
               ident_m):
    b, hh = c // 2, c % 2
    wq = w_qkv[0 * C + hh * 384: 0 * C + hh * 384 + 384].reshape(HL, D, C)
    wk = w_qkv[1 * C + hh * 384: 1 * C + hh * 384 + 384].reshape(HL, D, C)
    wv = w_qkv[2 * C + hh * 384: 2 * C + hh * 384 + 384]
    wq = wq[:, _PERM64, :].reshape(HL * D, C)
    wk = wk[:, _PERM64, :].reshape(HL * D, C)
    w_local = np.concatenate([wq, wk, wv], 0)       # (1152, 768)
    return {
        "xT": _bf16(x[b].T),
        "wqkvT": _bf16(w_local.T),
        "wpT": _bf16(w_proj[:, hh * 384: hh * 384 + 384].T),
        "sinr": sin_rep,
        "cosr": cos_rep,
        "negid": negid_m,
        "triu512": triu_m,
        "ident": ident_m,
    }


def kernel(x, w_qkv, w_proj, rope_sin, rope_cos, _trace=False):
    global _CACHED_NC
    x = np.asarray(x, dtype=np.float32)
    w_qkv = np.asarray(w_qkv, dtype=np.float32)
    w_proj = np.asarray(w_proj, dtype=np.float32)
    rope_sin = np.asarray(rope_sin, dtype=np.float32)
    rope_cos = np.asarray(rope_cos, dtype=np.float32)

    # (T, 384): per head block [table(32) | table(32)]
    sin_rep = _bf16(np.tile(np.concatenate([rope_sin, rope_sin], 1), (1, HL)))
    cos_rep = _bf16(np.tile(np.concatenate([rope_cos, rope_cos], 1), (1, HL)))
    negid_m = _bf16(np.eye(128) * -1e5)
    # strict upper triangle (key > query -> masked), zero-padded to 512
    triu_m = np.zeros((128, 512), np.float32)
    triu_m[:, 0:128] = np.arange(128)[:, None] > np.arange(128)[None, :]
    triu_m = _bf16(triu_m)
    ident_m = _bf16(np.eye(128))

    in_maps = [_prep_core(c, x, w_qkv, w_proj, sin_rep, cos_rep, negid_m,
                          triu_m, ident_m)
               for c in range(NCORES)]

    if _CACHED_NC is None:
        _CACHED_NC = build_nc()
    nc = _CACHED_NC

    try:
        res = run_bass_kernel_spmd(nc, in_maps, core_ids=list(range(NCORES)),
                                   trace=_trace)
    except Exception:
        # transient NRT_EXEC_UNIT_UNRECOVERABLE has been observed once per
        # ~20 runs in this environment; a single retry always recovered
        res = run_bass_kernel_spmd(nc, in_maps, core_ids=list(range(NCORES)),
                                   trace=_trace)
    parts = [res.results[c]["out"].astype(np.float32) for c in range(NCORES)]
    out = np.stack([parts[2 * b] + parts[2 * b + 1] for b in range(B)], 0)
    if _trace:
        return out.astype(np.float32), res
    return out.astype(np.float32)

